# revision 24
# baseline (speedup 1.0000x reference)
"""Canny edge detector (nn_CannyDetector) — Trainium2 Bass kernel, 8 cores.

Sharding: spatial bands. Core k owns image rows [128k, 128k+128) of ALL 4
images (the reference's flat-index NMS gather couples all 4 images at each
pixel). Window 0 = 110 output rows, per-image; window 1 = the last 18 rows
of the band with ALL FOUR images packed on partition blocks 32b (22 rows
each), so its elementwise pipeline runs once instead of 4x.

Perf notes (cost model): elementwise op cost ~ free-dim size only; DVE
tensor_scalar gets 2x(f32)/4x(f16) perf modes, tensor_tensor gets 2x only
when all operands are 2-byte; scalar_tensor_tensor never gets modes.
Row (dy) shifts are DMA SBUF->SBUF partition-shifted copies (the one
garbage edge row each only feeds rows that are never stored). The packed
window's cross-image compare operands (m_J replicated / shifted by the
per-image NMS direction) are likewise built with 4 block-DMAs each. ALL
conv arithmetic (A, PQ, magnitudes m, gxa/gya) must be exact f32: the NMS
strict compares amplify rounding (6.6e-5 noise => ~13k output flips; the
2e-2 gate allows ~420; f32r and f16 upstream both fail). Masks/flags are
f16 (exact 0/1). Hysteresis gate via ACT Sign(c3 - 0.5). Output f16
{0,1} -> f32 on host.
"""
import sys
import numpy as np

if "/opt/trn_rl_repo" not in sys.path:
    sys.path.insert(0, "/opt/trn_rl_repo")

# ---------------- geometry ----------------
B, C, H, W = 4, 3, 1024, 1024
NCORES = 8
BAND = H // NCORES              # 128 rows per core
HALO = 9
SLABR = BAND + 2 * HALO         # 146 input rows per core
WP = 1056                       # padded width: 6 left zeros, 26 right zeros
WINS = [(0, 110), (110, 18)]    # (start, R) output row windows within band
DIRS = [(0, 1), (1, 1), (1, 0), (1, -1)]   # d_b for b = 0..3 (E, SE, S, SW)

_cache = {}


def _build():
    import concourse.bass as bass
    import concourse.tile as tile
    from concourse import bacc, mybir
    from contextlib import ExitStack

    F32 = mybir.dt.float32
    F16 = mybir.dt.float16
    AF = mybir.ActivationFunctionType
    OP = mybir.AluOpType

    nc = bacc.Bacc("TRN2", target_bir_lowering=False, debug=False,
                   num_devices=NCORES)
    xT = nc.dram_tensor("xT", [B * C, WP, SLABR], F32, kind="ExternalInput").ap()
    bandHL = nc.dram_tensor("bandHL", [128, 128], F32,
                            kind="ExternalInput").ap()
    bandHR = nc.dram_tensor("bandHR", [128, 12], F32,
                            kind="ExternalInput").ap()
    # per-core sobel-vertical bands (mask folded): w0 P|Q [128, 228], w1 [36, 44]
    bandPQ0 = nc.dram_tensor("bandPQ0", [128, 228], F32,
                             kind="ExternalInput").ap()
    bandPQ1 = nc.dram_tensor("bandPQ1", [36, 44], F32,
                             kind="ExternalInput").ap()
    # f16 [1,1,1] vertical band | -I  [128, 256] (integer sums — exact);
    # bandC3P is the block-diagonal variant for the packed window
    bandC3 = nc.dram_tensor("bandC3", [128, 256], F16,
                            kind="ExternalInput").ap()
    bandC3P = nc.dram_tensor("bandC3P", [128, 256], F16,
                             kind="ExternalInput").ap()
    aux = nc.dram_tensor("aux", [128, 8], F32, kind="ExternalInput").ap()
    out = nc.dram_tensor("out", [B, BAND, W], F16, kind="ExternalOutput").ap()

    with tile.TileContext(nc) as tc, ExitStack() as ctx:
        dve, gp, act = nc.vector, nc.gpsimd, nc.scalar

        consts = ctx.enter_context(tc.tile_pool(name="consts", bufs=1))
        xcp = ctx.enter_context(tc.tile_pool(name="xc", bufs=2))
        psa = ctx.enter_context(tc.tile_pool(name="psa", bufs=2, space="PSUM"))
        psb = ctx.enter_context(tc.tile_pool(name="psb", bufs=2, space="PSUM"))
        psc = ctx.enter_context(tc.tile_pool(name="psc", bufs=1, space="PSUM"))
        hbp = ctx.enter_context(tc.tile_pool(name="hbp", bufs=2))
        gxyp = ctx.enter_context(tc.tile_pool(name="gxyp", bufs=4))
        mmp = ctx.enter_context(tc.tile_pool(name="mmp", bufs=1))
        grp = ctx.enter_context(tc.tile_pool(name="grp", bufs=1))
        ded = ctx.enter_context(tc.tile_pool(name="ded", bufs=1))
        scr = ctx.enter_context(tc.tile_pool(name="scr", bufs=5))
        sch = ctx.enter_context(tc.tile_pool(name="sch", bufs=2))
        pqp = ctx.enter_context(tc.tile_pool(name="pqp", bufs=3))
        scb = ctx.enter_context(tc.tile_pool(name="scb", bufs=6))

        bHL = consts.tile([128, 128], F32)
        nc.sync.dma_start(bHL[:], bandHL[:])
        bHR = consts.tile([128, 12], F32)
        nc.sync.dma_start(bHR[:], bandHR[:])
        bPQ0 = consts.tile([128, 228], F32)
        nc.sync.dma_start(bPQ0[:], bandPQ0[:])
        bPQ1 = consts.tile([36, 44], F32)
        nc.sync.dma_start(bPQ1[:], bandPQ1[:])
        bC3 = consts.tile([128, 256], F16)
        nc.sync.dma_start(bC3[:], bandC3[:])
        bC3P = consts.tile([128, 256], F16)
        nc.sync.dma_start(bC3P[:], bandC3P[:])
        auxt = consts.tile([128, 8], F32)
        nc.sync.dma_start(auxt[:], aux[:])

        TAN1 = float(np.float32(np.tan(np.pi / 8)))
        TAN3 = float(np.float32(np.tan(3 * np.pi / 8)))

        # pad-column memsets are only needed for the first `bufs` rotations
        # of a pool slot: data writes never touch the pads afterwards
        _seen = {}

        def pad_once(key, bufs):
            n = _seen.get(key, 0)
            _seen[key] = n + 1
            return n < bufs

        def phase_a(ci, wst, Rin):
            """13-tap horizontal gauss for (image,channel) ci on this row
            window -> hb [Rin, 1026] f32 (cols 1..1024 valid)."""
            xall = xcp.tile([128, 9 * 128], F32, tag="xall")
            xv = xall[:, 0:9 * Rin].rearrange("p (c r) -> p c r", c=9)
            nc.sync.dma_start(
                xv[:, 0:8, :],
                xT[ci, 0:1024, wst:wst + Rin].rearrange(
                    "(c p) r -> p c r", p=128))
            nc.sync.dma_start(
                xv[0:32, 8, :], xT[ci, 1024:1056, wst:wst + Rin])
            hb = hbp.tile([128, 1026], F32, tag="hblur")
            for g in range(2):          # out blocks {0-3}, {4-7}
                pt = psa.tile([128, 512], F32, tag="psa")
                for jj in range(4):
                    j = 4 * g + jj
                    nc.tensor.matmul(
                        pt[0:Rin, 128 * jj:128 * jj + 128],
                        xv[:, j, :], bHL[:, :],
                        start=True, stop=True)
                    rk = 32 if j == 7 else 128
                    nc.tensor.matmul(
                        pt[0:Rin, 128 * jj + 116:128 * jj + 128],
                        xv[0:rk, j + 1, :], bHR[0:rk, :],
                        start=False, stop=True)
                act.copy(hb[0:Rin, 1 + 512 * g:513 + 512 * g],
                         pt[0:Rin, 0:512])
            return hb

        def grad_mag(NPr, pqP, qqP, mMa, sprev, c, m_dst, fixup=None):
            """p/q psum -> gx, gy, and magnitude accumulation (rows [0:NPr]).
            Returns (gxt, gyt, new sprev)."""
            psb_ = pqp.tile([128, 1026], F32, tag="pqs")
            if pad_once("pqs", 3):
                gp.memset(psb_[:, 0:1], 0.0)
                gp.memset(psb_[:, 1025:1026], 0.0)
            act.copy(psb_[0:NPr, 1:1025], pqP[0:NPr, :])
            qsb = pqp.tile([128, 1026], F32, tag="pqs")
            if pad_once("pqs", 3):
                gp.memset(qsb[:, 0:1], 0.0)
                gp.memset(qsb[:, 1025:1026], 0.0)
            act.copy(qsb[0:NPr, 1:1025], qqP[0:NPr, :])
            if fixup is not None:
                fixup(psb_, qsb)
            # gx = p[w-1] - p[w+1]
            gxt = gxyp.tile([128, 1024], F32, tag="gx")
            dve.tensor_tensor(gxt[0:NPr, :], psb_[0:NPr, 0:1024],
                              psb_[0:NPr, 2:1026], OP.subtract)
            # gy = q[w-1] + 2q[w] + q[w+1]
            gyt = gxyp.tile([128, 1024], F32, tag="gy")
            dve.scalar_tensor_tensor(gyt[0:NPr, :], qsb[0:NPr, 1:1025],
                                     2.0, qsb[0:NPr, 0:1024],
                                     OP.mult, OP.add)
            gp.tensor_tensor(gyt[0:NPr, :], gyt[0:NPr, :],
                             qsb[0:NPr, 2:1026], OP.add)
            # magnitude
            sx = scr.tile([128, 1024], F32, tag="scr")
            act.activation(sx[0:NPr, :], gxt[0:NPr, :], AF.Square)
            u = scr.tile([128, 1024], F32, tag="scr")
            act.activation(u[0:NPr, :], gyt[0:NPr, :], AF.Square)
            gp.tensor_tensor(u[0:NPr, :], u[0:NPr, :], sx[0:NPr, :],
                             OP.add)
            sq = scr.tile([128, 1024], F32, tag="scr")
            act.activation(sq[0:NPr, :], u[0:NPr, :], AF.Sqrt, scale=mMa)
            if c == 0:
                return gxt, gyt, sq
            if c == 1:
                s01 = scr.tile([128, 1024], F32, tag="scr")
                dve.tensor_tensor(s01[0:NPr, :], sprev[0:NPr, :],
                                  sq[0:NPr, :], OP.add)
                return gxt, gyt, s01
            dve.tensor_tensor(m_dst, sprev[0:NPr, :], sq[0:NPr, :], OP.add)
            return gxt, gyt, None

        def grad_sums(NPr, gxa, gya, gxc, gyc):
            gp.tensor_tensor(gxa[0:NPr, :], gxc[0][0:NPr, :],
                             gxc[1][0:NPr, :], OP.add)
            dve.tensor_tensor(gxa[0:NPr, :], gxa[0:NPr, :],
                              gxc[2][0:NPr, :], OP.add)
            gp.tensor_tensor(gya[0:NPr, :], gyc[0][0:NPr, :],
                             gyc[1][0:NPr, :], OP.add)
            dve.tensor_tensor(gya[0:NPr, :], gya[0:NPr, :],
                              gyc[2][0:NPr, :], OP.add)

        def nms(NPr, gxa, gya, get_cmp, mb, LOWa, HIGHa, mTa, c3b,
                emit_store):
            """NMS + hysteresis on rows [0:NPr]. get_cmp(i, sg) -> (lhs,
            rhs) f32 APs for the strict compare of image i's magnitudes
            against its sg*d-shifted neighbors. mb: magnitude AP for the
            threshold tests. c3b: the [1,1,1]v|-I band tile."""
            ax = sch.tile([128, 1024], F32, tag="scrf")
            act.activation(ax[0:NPr, :], gxa[0:NPr, :], AF.Abs)
            ay = sch.tile([128, 1024], F32, tag="scrf")
            act.activation(ay[0:NPr, :], gya[0:NPr, :], AF.Abs)
            c1 = ded.tile([128, 1024], F16, tag="c1")
            dve.scalar_tensor_tensor(c1[0:NPr, :], ax[0:NPr, :], TAN1,
                                     ay[0:NPr, :], OP.mult, OP.is_ge)
            c2 = ded.tile([128, 1024], F16, tag="c2")
            dve.scalar_tensor_tensor(c2[0:NPr, :], ax[0:NPr, :], TAN3,
                                     ay[0:NPr, :], OP.mult, OP.is_lt)
            sp = sch.tile([128, 1024], F16, tag="scrh")
            dve.tensor_tensor(sp[0:NPr, :], gxa[0:NPr, :], gya[0:NPr, :],
                              OP.mult)
            pos = scb.tile([128, 1024], F16, tag="scb")
            dve.tensor_scalar(pos[0:NPr, :], sp[0:NPr, :], 0.0, None,
                              OP.is_gt)
            dg = scb.tile([128, 1024], F16, tag="scb")
            dve.tensor_tensor(dg[0:NPr, :], c1[0:NPr, :], c2[0:NPr, :],
                              OP.add)
            dve.tensor_scalar(dg[0:NPr, :], dg[0:NPr, :], -1.0, 1.0,
                              OP.mult, OP.add)
            dp = ded.tile([128, 1024], F16, tag="dp")
            dve.tensor_tensor(dp[0:NPr, :], dg[0:NPr, :], pos[0:NPr, :],
                              OP.mult)
            dn = ded.tile([128, 1024], F16, tag="dn")
            dve.tensor_tensor(dn[0:NPr, :], dg[0:NPr, :], dp[0:NPr, :],
                              OP.subtract)

            im = ded.tile([128, 1024], F16, tag="im")
            acc = None
            for pi, (mask, J, sg) in enumerate(
                    [(c1, 0, 1), (c2, 1, 1), (dp, 0, -1), (dn, 1, -1)]):
                pp = scb.tile([128, 1024], F16, tag="scb")
                pfirst = None
                for k, i in enumerate((J, J + 2)):
                    lhs, rhs = get_cmp(i, sg)
                    cmp_ = scb.tile([128, 1024], F16, tag="scb")
                    dve.tensor_tensor(cmp_[0:NPr, :], lhs, rhs, OP.is_gt)
                    if k == 0:
                        pfirst = cmp_
                    else:
                        dve.tensor_tensor(pp[0:NPr, :], pfirst[0:NPr, :],
                                          cmp_[0:NPr, :], OP.mult)
                t_ = scb.tile([128, 1024], F16, tag="scb")
                dve.tensor_tensor(t_[0:NPr, :], mask[0:NPr, :],
                                  pp[0:NPr, :], OP.mult)
                if acc is None:
                    acc = t_
                elif pi < 3:
                    a2 = scb.tile([128, 1024], F16, tag="scb")
                    dve.tensor_tensor(a2[0:NPr, :], acc[0:NPr, :],
                                      t_[0:NPr, :], OP.add)
                    acc = a2
                else:
                    dve.tensor_tensor(im[0:NPr, :], acc[0:NPr, :],
                                      t_[0:NPr, :], OP.add)
            mh = ded.tile([128, 1024], F16, tag="mh")
            dve.tensor_scalar(mh[0:NPr, :], mb, HIGHa, None, OP.is_gt)
            hp = ded.tile([128, 1026], F16, tag="hp")
            if pad_once("hp", 1):
                gp.memset(hp[:, 0:1], 0.0)
                gp.memset(hp[:, 1025:1026], 0.0)
            dve.tensor_tensor(hp[0:NPr, 1:1025], im[0:NPr, :],
                              mh[0:NPr, :], OP.mult)
            ml = scb.tile([128, 1024], F16, tag="scb")
            dve.tensor_scalar(ml[0:NPr, :], mb, LOWa, None, OP.is_ge)
            m1 = scb.tile([128, 1024], F16, tag="scb")
            dve.tensor_scalar(m1[0:NPr, :], mb, HIGHa, None, OP.is_le)
            mid = ded.tile([128, 1024], F16, tag="mid")
            gp.tensor_tensor(mid[0:NPr, :], ml[0:NPr, :], m1[0:NPr, :],
                             OP.mult)
            r3 = ded.tile([128, 1024], F16, tag="r3")
            dve.tensor_tensor(r3[0:NPr, :], hp[0:NPr, 0:1024],
                              hp[0:NPr, 2:1026], OP.add)
            dve.tensor_tensor(r3[0:NPr, :], r3[0:NPr, :],
                              hp[0:NPr, 1:1025], OP.add)
            # connect = [1,1,1]v @ r3 - hp  (f16 bands, integer-exact)
            c3p = psc.tile([128, 1024], F32, tag="psc")
            nc.tensor.matmul(c3p[0:NPr, 0:512], c3b[0:NPr, 0:NPr],
                             r3[0:NPr, 0:512], start=True, stop=True)
            nc.tensor.matmul(c3p[0:NPr, 512:1024], c3b[0:NPr, 0:NPr],
                             r3[0:NPr, 512:1024], start=True, stop=True)
            nc.tensor.matmul(c3p[0:NPr, 0:512], c3b[0:NPr, 128:128 + NPr],
                             hp[0:NPr, 1:513], start=False, stop=True)
            nc.tensor.matmul(c3p[0:NPr, 512:1024],
                             c3b[0:NPr, 128:128 + NPr],
                             hp[0:NPr, 513:1025], start=False, stop=True)
            # c3 is a nonneg integer; Sign(c3 - 0.5) = +1 iff c3 >= 1,
            # else -1 (negatives absorbed by the max with mh below)
            sgn = scb.tile([128, 1024], F16, tag="scb")
            act.activation(sgn[0:NPr, :], c3p[0:NPr, :], AF.Sign,
                           bias=auxt[0:NPr, 7:8])
            t_g = scb.tile([128, 1024], F16, tag="scb")
            dve.tensor_tensor(t_g[0:NPr, :], sgn[0:NPr, :], mid[0:NPr, :],
                              OP.mult)
            mx = scb.tile([128, 1024], F16, tag="scb")
            dve.tensor_tensor(mx[0:NPr, :], mh[0:NPr, :], t_g[0:NPr, :],
                              OP.max)
            # th = ((mx * im) * mT)
            th1 = scb.tile([128, 1024], F16, tag="scb")
            dve.tensor_tensor(th1[0:NPr, :], mx[0:NPr, :], im[0:NPr, :],
                              OP.mult)
            th = ded.tile([128, 1024], F16, tag="r3")
            dve.tensor_scalar(th[0:NPr, :], th1[0:NPr, :], mTa, None,
                              OP.mult)
            gp.memset(th[0:NPr, 0:1], 0.0)
            gp.memset(th[0:NPr, 1023:1024], 0.0)
            emit_store(th)

        # ================== window 0: 110 rows, per-image =================
        wst0, R0 = WINS[0]
        Rin0, R40 = R0 + 18, R0 + 4
        mM = auxt[0:R40, 4:5]
        bP = bPQ0[0:Rin0, 0:R40]
        bQ = bPQ0[0:Rin0, 114:114 + R40]

        m_t = [None] * B      # m maps [R40, 1026] f32 (col 0 / 1025 zero)
        mu_t = [None] * B
        md_t = [None] * B
        gxs_t = [None] * B
        gys_t = [None] * B

        for b in range(B):
            gxa = grp.tile([128, 1024], F32, tag=f"gxa{b}")
            gya = grp.tile([128, 1024], F32, tag=f"gya{b}")
            mt = mmp.tile([128, 1026], F32, tag=f"m{b}")
            if pad_once(f"m{b}", 1):
                gp.memset(mt[:, 0:1], 0.0)
                gp.memset(mt[:, 1025:1026], 0.0)
            sprev = None
            gxc = []
            gyc = []
            for c in range(C):
                hb = phase_a(b * C + c, wst0, Rin0)
                pq = psb.tile([128, 1024], F32, tag="pq")
                h1 = hb[0:Rin0, 1:513]
                h2_ = hb[0:Rin0, 513:1025]
                nc.tensor.matmul(pq[0:R40, 0:512], bP, h1,
                                 start=True, stop=True)
                nc.tensor.matmul(pq[0:R40, 512:1024], bP, h2_,
                                 start=True, stop=True)
                qq = psb.tile([128, 1024], F32, tag="pq")
                nc.tensor.matmul(qq[0:R40, 0:512], bQ, h1,
                                 start=True, stop=True)
                nc.tensor.matmul(qq[0:R40, 512:1024], bQ, h2_,
                                 start=True, stop=True)
                gxt, gyt, sprev = grad_mag(R40, pq, qq, mM, sprev, c,
                                           mt[0:R40, 1:1025])
                gxc.append(gxt)
                gyc.append(gyt)
            grad_sums(R40, gxa, gya, gxc, gyc)
            m_t[b] = mt
            gxs_t[b], gys_t[b] = gxa, gya

            # ---- mU / mD row shifts via SBUF->SBUF DMA ----
            # mu[r] = m[r+1] for r < R40-1; mu[R40-1] / md[0] are stale
            # garbage — they only feed NMS rows whose outputs are never
            # stored (th rows [2, 2+R0)), and is_gt of junk yields 0/1.
            mu = mmp.tile([128, 1026], F32, tag=f"mu{b}")
            nc.sync.dma_start(mu[0:R40 - 1, :], mt[1:R40, :])
            md = mmp.tile([128, 1026], F32, tag=f"md{b}")
            act.dma_start(md[1:R40, :], mt[0:R40 - 1, :])
            mu_t[b], md_t[b] = mu, md

        # -------------------- window 0 NMS (per image) -------------------
        for b in range(B):
            dy, dx = DIRS[b]

            def get_cmp(i, sg, dy=dy, dx=dx):
                src_ = {0: m_t, 1: mu_t, -1: md_t}[sg * dy][i]
                return (m_t[i][0:R40, 1:1025],
                        src_[0:R40, 1 + sg * dx:1 + sg * dx + 1024])

            def store(th, b=b):
                nc.sync.dma_start(out[b, wst0:wst0 + R0, 0:1024],
                                  th[2:2 + R0, 0:1024])

            nms(R40, gxs_t[b], gys_t[b], get_cmp, m_t[b][0:R40, 1:1025],
                auxt[0:R40, 0:1], auxt[0:R40, 1:2], auxt[0:R40, 2:3], bC3,
                store)

        # ========== window 1 convs: last 18 rows, 4 images packed ========
        # Emitted BEFORE the window-0 NMS so the in-order PE queue overlaps
        # this conv work with the DVE-heavy NMS phase.
        wst1, R1 = WINS[1]
        Rin, R4 = R1 + 18, R1 + 4
        NP = 96 + R4                      # 118 used partitions
        bP1 = bPQ1[0:Rin, 0:R4]
        bQ1 = bPQ1[0:Rin, 22:22 + R4]

        # m_p gap rows (32b+22..32b+31) come out exactly 0: the packed mM
        # scale is 0 there and the psum garbage under it is finite.
        m_p = mmp.tile([128, 1026], F32, tag="m0")
        sprev = None
        gxc = []
        gyc = []
        for c in range(C):
            pqP = psb.tile([128, 1024], F32, tag="pq")
            qqP = psb.tile([128, 1024], F32, tag="pq")
            s3 = None
            for b in range(B):
                hb = phase_a(b * C + c, wst1, Rin)
                h1 = hb[0:Rin, 1:513]
                h2_ = hb[0:Rin, 513:1025]
                if b < 3:
                    o = 32 * b
                    nc.tensor.matmul(pqP[o:o + R4, 0:512], bP1, h1,
                                     start=True, stop=True)
                    nc.tensor.matmul(pqP[o:o + R4, 512:1024], bP1, h2_,
                                     start=True, stop=True)
                    nc.tensor.matmul(qqP[o:o + R4, 0:512], bQ1, h1,
                                     start=True, stop=True)
                    nc.tensor.matmul(qqP[o:o + R4, 512:1024], bQ1, h2_,
                                     start=True, stop=True)
                else:
                    # PE out base must be 0/32/64: block 3 (partitions 96+)
                    # goes through its own psum tile + SBUF, then a DMA
                    # bounce into the packed p/q tiles.
                    p3 = psc.tile([128, 1024], F32, tag="psc")
                    nc.tensor.matmul(p3[0:R4, 0:512], bP1, h1,
                                     start=True, stop=True)
                    nc.tensor.matmul(p3[0:R4, 512:1024], bP1, h2_,
                                     start=True, stop=True)
                    nc.tensor.matmul(p3[32:32 + R4, 0:512], bQ1, h1,
                                     start=True, stop=True)
                    nc.tensor.matmul(p3[32:32 + R4, 512:1024], bQ1, h2_,
                                     start=True, stop=True)
                    s3 = sch.tile([128, 1024], F32, tag="scrf")
                    act.copy(s3[0:R4, :], p3[0:R4, :])
                    act.copy(s3[32:32 + R4, :], p3[32:32 + R4, :])

            def fixup(psb_, qsb, s3=s3):
                nc.sync.dma_start(psb_[96:96 + R4, 1:1025], s3[0:R4, :])
                nc.sync.dma_start(qsb[96:96 + R4, 1:1025],
                                  s3[32:32 + R4, :])

            gxt, gyt, sprev = grad_mag(NP, pqP, qqP, auxt[0:NP, 5:6],
                                       sprev, c, m_p[0:NP, 1:1025], fixup)
            gxc.append(gxt)
            gyc.append(gyt)
        gxap = grp.tile([128, 1024], F32, tag="gxa0")
        gyap = grp.tile([128, 1024], F32, tag="gya0")
        grad_sums(NP, gxap, gyap, gxc, gyc)

        # cross-image compare operands, built by partition/col-shifted DMAs:
        # rep[i] = m_i replicated into every 32-block; rhs[(i, s)] block b =
        # m_i shifted by s*d_b. Over/underflowing edge rows stay stale —
        # they only feed unstored output rows.
        rep = []
        for i in range(B):
            rp = mmp.tile([128, 1026], F32, tag=f"mu{i}")
            for b in range(B):
                qe = nc.sync if (i + b) % 2 == 0 else act
                qe.dma_start(rp[32 * b:32 * b + R4, 1:1025],
                             m_p[32 * i:32 * i + R4, 1:1025])
            rep.append(rp)
        rhs = {}
        rtags = {(0, 1): "md0", (1, 1): "md1", (2, 1): "md2",
                 (3, 1): "md3", (0, -1): "m1", (1, -1): "m2",
                 (2, -1): "m3"}
        for i in range(B):
            for s in (1, -1):
                if (i, s) in rtags:
                    rt = mmp.tile([128, 1026], F32, tag=rtags[(i, s)])
                else:
                    rt = pqp.tile([128, 1026], F32, tag="pqs")
                for b in range(B):
                    dyb, dxb = DIRS[b]
                    sp_ = 32 * i + s * dyb
                    lo = max(sp_, 0)
                    hi = min(sp_ + R4, 128)
                    qe = nc.sync if (i + b + (s > 0)) % 2 == 0 else act
                    qe.dma_start(
                        rt[32 * b + (lo - sp_):32 * b + (hi - sp_), 1:1025],
                        m_p[lo:hi, 1 + s * dxb:1025 + s * dxb])
                rhs[(i, s)] = rt

        def get_cmp_p(i, sg):
            return (rep[i][0:NP, 1:1025], rhs[(i, sg)][0:NP, 1:1025])

        def store_p(th):
            for b in range(B):
                nc.sync.dma_start(out[b, wst1:wst1 + R1, 0:1024],
                                  th[32 * b + 2:32 * b + 2 + R1, 0:1024])

        nms(NP, gxap, gyap, get_cmp_p, m_p[0:NP, 1:1025],
            auxt[0:NP, 0:1], auxt[0:NP, 1:2], auxt[0:NP, 3:4], bC3P,
            store_p)

    nc.compile()
    return nc


def _host_prep(img, gauss_h):
    """Build per-core inputs. Returns (in_maps, low, high)."""
    gh = np.asarray(gauss_h, np.float32).reshape(-1)

    flat = img.reshape(-1)
    r = (flat.size - 1) // 2
    v = np.partition(flat, r)[r]
    t1 = np.float32(max(np.float32(0.0),
                        np.float32(np.float32(0.7) * v)) * np.float32(6.0))
    t2 = np.float32(min(np.float32(1.0),
                        np.float32(np.float32(1.3) * v)) * np.float32(6.0))
    low = np.float32(min(t1, t2))
    high = np.float32(max(t1, t2))

    p = np.arange(128)[:, None]
    n = np.arange(128)[None, :]
    t = p - n
    bandHL = np.where((t >= 0) & (t <= 12), gh[np.clip(t, 0, 12)], 0.0
                      ).astype(np.float32)
    q12 = np.arange(12)[None, :]
    t12 = 12 + p - q12
    bandHR = np.where((t12 >= 0) & (t12 <= 12), gh[np.clip(t12, 0, 12)], 0.0
                      ).astype(np.float32)

    t5 = np.arange(128)[:, None] - n
    c111 = np.where(np.abs(t5) <= 1, 1.0, 0.0).astype(np.float32)
    negI = np.where(t5 == 0, -1.0, 0.0).astype(np.float32)
    bandC3 = np.concatenate([c111, negI], axis=1).astype(np.float16)
    # packed variant: [1,1,1] only within each 32-block's first 22 rows
    blk = np.arange(128) // 32
    rw = np.arange(128) % 32
    sameb = (blk[:, None] == blk[None, :])
    vq = (rw < 22)
    c111p = np.where(sameb & (np.abs(t5) <= 1)
                     & vq[:, None] & vq[None, :], 1.0, 0.0)
    bandC3P = np.concatenate([c111p, negI], axis=1).astype(np.float16)

    padded = np.zeros((B, C, H + 2 * HALO, W), np.float32)
    padded[:, :, HALO:HALO + H, :] = img

    w121 = np.array([1.0, 2.0, 1.0], np.float32)
    w101 = np.array([1.0, 0.0, -1.0], np.float32)

    in_maps = []
    for k in range(NCORES):
        slab = padded[:, :, BAND * k:BAND * k + SLABR, :]  # [B, C, SLABR, W]
        xT = np.zeros((B * C, WP, SLABR), np.float32)
        xT[:, 6:6 + W, :] = slab.reshape(B * C, SLABR, W).transpose(0, 2, 1)
        aux = np.zeros((128, 8), np.float32)
        aux[:, 0] = low
        aux[:, 1] = high
        aux[:, 7] = -0.5
        pq = []
        for wi, (wst, R) in enumerate(WINS):
            Rin, R4, R6 = R + 18, R + 4, R + 6
            g0 = BAND * k + wst
            maskBV = np.array([1.0 if 0 <= g0 - 3 + i < H else 0.0
                               for i in range(R6)], np.float32)
            for i in range(R4):
                mMv = 1.0 if 0 <= g0 - 2 + i < H else 0.0
                gr = g0 - 2 + i
                mTv = 0.0 if (gr == 0 or gr == H - 1) else 1.0
                if wi == 0:
                    aux[i, 4] = mMv
                    aux[i, 2] = mTv
                else:       # packed layout: same values in every 32-block
                    for b in range(B):
                        aux[32 * b + i, 5] = mMv
                        aux[32 * b + i, 3] = mTv
            # bandP[p, m] = sum_t w121[t] * maskBV[m+t] * gv[p-m-t]
            bP = np.zeros((Rin, R4), np.float32)
            bQ = np.zeros((Rin, R4), np.float32)
            pp_ = np.arange(Rin)[:, None]
            mm_ = np.arange(R4)[None, :]
            for ti in range(3):
                idx = pp_ - mm_ - ti
                gvv = np.where((idx >= 0) & (idx <= 12),
                               gh[np.clip(idx, 0, 12)], 0.0)
                bP += np.float32(w121[ti]) * maskBV[None, mm_[0] + ti] * gvv
                bQ += np.float32(w101[ti]) * maskBV[None, mm_[0] + ti] * gvv
            pq.append((bP.astype(np.float32), bQ.astype(np.float32)))
        b0 = np.zeros((128, 228), np.float32)
        b0[:, 0:114] = pq[0][0]
        b0[:, 114:228] = pq[0][1]
        b1 = np.zeros((36, 44), np.float32)
        b1[:, 0:22] = pq[1][0]
        b1[:, 22:44] = pq[1][1]
        in_maps.append({"xT": xT, "bandHL": bandHL, "bandHR": bandHR,
                        "bandPQ0": b0, "bandPQ1": b1,
                        "bandC3": bandC3, "bandC3P": bandC3P, "aux": aux})
    return in_maps, low, high


def kernel(img, gauss_h, gauss_v, sobel_h, sobel_v, dir_f, conn_f):
    from concourse import bass_utils

    img = np.ascontiguousarray(np.asarray(img, np.float32))
    in_maps, low, high = _host_prep(img, gauss_h)

    if "nc" not in _cache:
        _cache["nc"] = _build()
    nc = _cache["nc"]

    res = bass_utils.run_bass_kernel_spmd(
        nc, in_maps, core_ids=list(range(NCORES)))
    outs = [np.asarray(res.results[k]["out"], np.float32)
            for k in range(NCORES)]
    full = np.concatenate(outs, axis=1)          # [B, H, W]
    return full[:, None, :, :].astype(np.float32)


# revision 25
# speedup vs baseline: 1.0075x; 1.0075x over previous
"""Canny edge detector (nn_CannyDetector) — Trainium2 Bass kernel, 8 cores.

Sharding: spatial bands. Core k owns image rows [128k, 128k+128) of ALL 4
images (the reference's flat-index NMS gather couples all 4 images at each
pixel). Window 0 = 110 output rows, per-image; window 1 = the last 18 rows
of the band with ALL FOUR images packed on partition blocks 32b (22 rows
each), so its elementwise pipeline runs once instead of 4x.

Perf notes (cost model): elementwise op cost ~ free-dim size only; DVE
tensor_scalar gets 2x(f32)/4x(f16) perf modes, tensor_tensor gets 2x only
when all operands are 2-byte; scalar_tensor_tensor never gets modes.
Row (dy) shifts are DMA SBUF->SBUF partition-shifted copies (the one
garbage edge row each only feeds rows that are never stored). The packed
window's cross-image compare operands (m_J replicated / shifted by the
per-image NMS direction) are likewise built with 4 block-DMAs each. ALL
conv arithmetic (A, PQ, magnitudes m, gxa/gya) must be exact f32: the NMS
strict compares amplify rounding (6.6e-5 noise => ~13k output flips; the
2e-2 gate allows ~420; f32r and f16 upstream both fail). Masks/flags are
f16 (exact 0/1). Hysteresis gate via ACT Sign(c3 - 0.5). Output f16
{0,1} -> f32 on host.
"""
import sys
import numpy as np

if "/opt/trn_rl_repo" not in sys.path:
    sys.path.insert(0, "/opt/trn_rl_repo")

# ---------------- geometry ----------------
B, C, H, W = 4, 3, 1024, 1024
NCORES = 8
BAND = H // NCORES              # 128 rows per core
HALO = 9
SLABR = BAND + 2 * HALO         # 146 input rows per core
WP = 1056                       # padded width: 6 left zeros, 26 right zeros
WINS = [(0, 110), (110, 18)]    # (start, R) output row windows within band
DIRS = [(0, 1), (1, 1), (1, 0), (1, -1)]   # d_b for b = 0..3 (E, SE, S, SW)

_cache = {}


def _build():
    import concourse.bass as bass
    import concourse.tile as tile
    from concourse import bacc, mybir
    from contextlib import ExitStack

    F32 = mybir.dt.float32
    F16 = mybir.dt.float16
    AF = mybir.ActivationFunctionType
    OP = mybir.AluOpType

    nc = bacc.Bacc("TRN2", target_bir_lowering=False, debug=False,
                   num_devices=NCORES)
    xT = nc.dram_tensor("xT", [B * C, WP, SLABR], F32, kind="ExternalInput").ap()
    bandHL = nc.dram_tensor("bandHL", [128, 128], F32,
                            kind="ExternalInput").ap()
    bandHR = nc.dram_tensor("bandHR", [128, 12], F32,
                            kind="ExternalInput").ap()
    # per-core sobel-vertical bands (mask folded): w0 P|Q [128, 228], w1 [36, 44]
    bandPQ0 = nc.dram_tensor("bandPQ0", [128, 228], F32,
                             kind="ExternalInput").ap()
    bandPQ1 = nc.dram_tensor("bandPQ1", [36, 44], F32,
                             kind="ExternalInput").ap()
    # f16 [1,1,1] vertical band | -I  [128, 256] (integer sums — exact);
    # bandC3P is the block-diagonal variant for the packed window
    bandC3 = nc.dram_tensor("bandC3", [128, 256], F16,
                            kind="ExternalInput").ap()
    bandC3P = nc.dram_tensor("bandC3P", [128, 256], F16,
                             kind="ExternalInput").ap()
    aux = nc.dram_tensor("aux", [128, 8], F32, kind="ExternalInput").ap()
    out = nc.dram_tensor("out", [B, BAND, W], F16, kind="ExternalOutput").ap()

    with tile.TileContext(nc) as tc, ExitStack() as ctx:
        dve, gp, act = nc.vector, nc.gpsimd, nc.scalar

        consts = ctx.enter_context(tc.tile_pool(name="consts", bufs=1))
        xcp = ctx.enter_context(tc.tile_pool(name="xc", bufs=2))
        psa = ctx.enter_context(tc.tile_pool(name="psa", bufs=2, space="PSUM"))
        psb = ctx.enter_context(tc.tile_pool(name="psb", bufs=2, space="PSUM"))
        psc = ctx.enter_context(tc.tile_pool(name="psc", bufs=1, space="PSUM"))
        hbp = ctx.enter_context(tc.tile_pool(name="hbp", bufs=2))
        gxyp = ctx.enter_context(tc.tile_pool(name="gxyp", bufs=4))
        mmp = ctx.enter_context(tc.tile_pool(name="mmp", bufs=1))
        grp = ctx.enter_context(tc.tile_pool(name="grp", bufs=1))
        ded = ctx.enter_context(tc.tile_pool(name="ded", bufs=1))
        scr = ctx.enter_context(tc.tile_pool(name="scr", bufs=5))
        sch = ctx.enter_context(tc.tile_pool(name="sch", bufs=2))
        pqp = ctx.enter_context(tc.tile_pool(name="pqp", bufs=3))
        scb = ctx.enter_context(tc.tile_pool(name="scb", bufs=6))

        bHL = consts.tile([128, 128], F32)
        nc.sync.dma_start(bHL[:], bandHL[:])
        bHR = consts.tile([128, 12], F32)
        nc.sync.dma_start(bHR[:], bandHR[:])
        bPQ0 = consts.tile([128, 228], F32)
        nc.sync.dma_start(bPQ0[:], bandPQ0[:])
        bPQ1 = consts.tile([36, 44], F32)
        nc.sync.dma_start(bPQ1[:], bandPQ1[:])
        bC3 = consts.tile([128, 256], F16)
        nc.sync.dma_start(bC3[:], bandC3[:])
        bC3P = consts.tile([128, 256], F16)
        nc.sync.dma_start(bC3P[:], bandC3P[:])
        auxt = consts.tile([128, 8], F32)
        nc.sync.dma_start(auxt[:], aux[:])

        TAN1 = float(np.float32(np.tan(np.pi / 8)))
        TAN3 = float(np.float32(np.tan(3 * np.pi / 8)))

        # pad-column memsets are only needed for the first `bufs` rotations
        # of a pool slot: data writes never touch the pads afterwards
        _seen = {}

        def pad_once(key, bufs):
            n = _seen.get(key, 0)
            _seen[key] = n + 1
            return n < bufs

        def phase_a(ci, wst, Rin):
            """13-tap horizontal gauss for (image,channel) ci on this row
            window -> hb [Rin, 1026] f32 (cols 1..1024 valid)."""
            xall = xcp.tile([128, 9 * 128], F32, tag="xall")
            xv = xall[:, 0:9 * Rin].rearrange("p (c r) -> p c r", c=9)
            nc.sync.dma_start(
                xv[:, 0:8, :],
                xT[ci, 0:1024, wst:wst + Rin].rearrange(
                    "(c p) r -> p c r", p=128))
            nc.sync.dma_start(
                xv[0:32, 8, :], xT[ci, 1024:1056, wst:wst + Rin])
            hb = hbp.tile([128, 1026], F32, tag="hblur")
            for g in range(2):          # out blocks {0-3}, {4-7}
                pt = psa.tile([128, 512], F32, tag="psa")
                for jj in range(4):
                    j = 4 * g + jj
                    nc.tensor.matmul(
                        pt[0:Rin, 128 * jj:128 * jj + 128],
                        xv[:, j, :], bHL[:, :],
                        start=True, stop=True)
                    rk = 32 if j == 7 else 128
                    nc.tensor.matmul(
                        pt[0:Rin, 128 * jj + 116:128 * jj + 128],
                        xv[0:rk, j + 1, :], bHR[0:rk, :],
                        start=False, stop=True)
                act.copy(hb[0:Rin, 1 + 512 * g:513 + 512 * g],
                         pt[0:Rin, 0:512])
            return hb

        def grad_mag(NPr, pqP, qqP, mMa, sprev, c, m_dst, fixup=None):
            """p/q psum -> gx, gy, and magnitude accumulation (rows [0:NPr]).
            Returns (gxt, gyt, new sprev)."""
            psb_ = pqp.tile([128, 1026], F32, tag="pqs")
            if pad_once("pqs", 3):
                gp.memset(psb_[:, 0:1], 0.0)
                gp.memset(psb_[:, 1025:1026], 0.0)
            act.copy(psb_[0:NPr, 1:1025], pqP[0:NPr, :])
            qsb = pqp.tile([128, 1026], F32, tag="pqs")
            if pad_once("pqs", 3):
                gp.memset(qsb[:, 0:1], 0.0)
                gp.memset(qsb[:, 1025:1026], 0.0)
            act.copy(qsb[0:NPr, 1:1025], qqP[0:NPr, :])
            if fixup is not None:
                fixup(psb_, qsb)
            # gx = p[w-1] - p[w+1]
            gxt = gxyp.tile([128, 1024], F32, tag="gx")
            dve.tensor_tensor(gxt[0:NPr, :], psb_[0:NPr, 0:1024],
                              psb_[0:NPr, 2:1026], OP.subtract)
            # gy = q[w-1] + 2q[w] + q[w+1]
            gyt = gxyp.tile([128, 1024], F32, tag="gy")
            dve.scalar_tensor_tensor(gyt[0:NPr, :], qsb[0:NPr, 1:1025],
                                     2.0, qsb[0:NPr, 0:1024],
                                     OP.mult, OP.add)
            gp.tensor_tensor(gyt[0:NPr, :], gyt[0:NPr, :],
                             qsb[0:NPr, 2:1026], OP.add)
            # magnitude
            sx = scr.tile([128, 1024], F32, tag="scr")
            act.activation(sx[0:NPr, :], gxt[0:NPr, :], AF.Square)
            u = scr.tile([128, 1024], F32, tag="scr")
            act.activation(u[0:NPr, :], gyt[0:NPr, :], AF.Square)
            gp.tensor_tensor(u[0:NPr, :], u[0:NPr, :], sx[0:NPr, :],
                             OP.add)
            sq = scr.tile([128, 1024], F32, tag="scr")
            act.activation(sq[0:NPr, :], u[0:NPr, :], AF.Sqrt, scale=mMa)
            if c == 0:
                return gxt, gyt, sq
            if c == 1:
                s01 = scr.tile([128, 1024], F32, tag="scr")
                dve.tensor_tensor(s01[0:NPr, :], sprev[0:NPr, :],
                                  sq[0:NPr, :], OP.add)
                return gxt, gyt, s01
            dve.tensor_tensor(m_dst, sprev[0:NPr, :], sq[0:NPr, :], OP.add)
            return gxt, gyt, None

        def grad_sums(NPr, gxa, gya, gxc, gyc):
            gp.tensor_tensor(gxa[0:NPr, :], gxc[0][0:NPr, :],
                             gxc[1][0:NPr, :], OP.add)
            dve.tensor_tensor(gxa[0:NPr, :], gxa[0:NPr, :],
                              gxc[2][0:NPr, :], OP.add)
            gp.tensor_tensor(gya[0:NPr, :], gyc[0][0:NPr, :],
                             gyc[1][0:NPr, :], OP.add)
            dve.tensor_tensor(gya[0:NPr, :], gya[0:NPr, :],
                              gyc[2][0:NPr, :], OP.add)

        def nms(NPr, gxa, gya, get_cmp, mb, LOWa, HIGHa, mTa, c3b,
                emit_store):
            """NMS + hysteresis on rows [0:NPr]. get_cmp(i, sg) -> (lhs,
            rhs) f32 APs for the strict compare of image i's magnitudes
            against its sg*d-shifted neighbors. mb: magnitude AP for the
            threshold tests. c3b: the [1,1,1]v|-I band tile."""
            ax = sch.tile([128, 1024], F32, tag="scrf")
            act.activation(ax[0:NPr, :], gxa[0:NPr, :], AF.Abs)
            ay = sch.tile([128, 1024], F32, tag="scrf")
            act.activation(ay[0:NPr, :], gya[0:NPr, :], AF.Abs)
            c1 = ded.tile([128, 1024], F16, tag="c1")
            dve.scalar_tensor_tensor(c1[0:NPr, :], ax[0:NPr, :], TAN1,
                                     ay[0:NPr, :], OP.mult, OP.is_ge)
            c2 = ded.tile([128, 1024], F16, tag="c2")
            dve.scalar_tensor_tensor(c2[0:NPr, :], ax[0:NPr, :], TAN3,
                                     ay[0:NPr, :], OP.mult, OP.is_lt)
            sp = sch.tile([128, 1024], F16, tag="scrh")
            dve.tensor_tensor(sp[0:NPr, :], gxa[0:NPr, :], gya[0:NPr, :],
                              OP.mult)
            pos = scb.tile([128, 1024], F16, tag="scb")
            dve.tensor_scalar(pos[0:NPr, :], sp[0:NPr, :], 0.0, None,
                              OP.is_gt)
            dg = scb.tile([128, 1024], F16, tag="scb")
            gp.tensor_tensor(dg[0:NPr, :], c1[0:NPr, :], c2[0:NPr, :],
                             OP.add)
            dve.tensor_scalar(dg[0:NPr, :], dg[0:NPr, :], -1.0, 1.0,
                              OP.mult, OP.add)
            dp = ded.tile([128, 1024], F16, tag="dp")
            dve.tensor_tensor(dp[0:NPr, :], dg[0:NPr, :], pos[0:NPr, :],
                              OP.mult)
            dn = ded.tile([128, 1024], F16, tag="dn")
            dve.tensor_tensor(dn[0:NPr, :], dg[0:NPr, :], dp[0:NPr, :],
                              OP.subtract)

            im = ded.tile([128, 1024], F16, tag="im")
            acc = None
            for pi, (mask, J, sg) in enumerate(
                    [(c1, 0, 1), (c2, 1, 1), (dp, 0, -1), (dn, 1, -1)]):
                pp = scb.tile([128, 1024], F16, tag="scb")
                pfirst = None
                for k, i in enumerate((J, J + 2)):
                    lhs, rhs = get_cmp(i, sg)
                    cmp_ = scb.tile([128, 1024], F16, tag="scb")
                    dve.tensor_tensor(cmp_[0:NPr, :], lhs, rhs, OP.is_gt)
                    if k == 0:
                        pfirst = cmp_
                    else:
                        dve.tensor_tensor(pp[0:NPr, :], pfirst[0:NPr, :],
                                          cmp_[0:NPr, :], OP.mult)
                t_ = scb.tile([128, 1024], F16, tag="scb")
                dve.tensor_tensor(t_[0:NPr, :], mask[0:NPr, :],
                                  pp[0:NPr, :], OP.mult)
                if acc is None:
                    acc = t_
                elif pi < 3:
                    a2 = scb.tile([128, 1024], F16, tag="scb")
                    dve.tensor_tensor(a2[0:NPr, :], acc[0:NPr, :],
                                      t_[0:NPr, :], OP.add)
                    acc = a2
                else:
                    dve.tensor_tensor(im[0:NPr, :], acc[0:NPr, :],
                                      t_[0:NPr, :], OP.add)
            mh = ded.tile([128, 1024], F16, tag="mh")
            dve.tensor_scalar(mh[0:NPr, :], mb, HIGHa, None, OP.is_gt)
            hp = ded.tile([128, 1026], F16, tag="hp")
            if pad_once("hp", 1):
                gp.memset(hp[:, 0:1], 0.0)
                gp.memset(hp[:, 1025:1026], 0.0)
            dve.tensor_tensor(hp[0:NPr, 1:1025], im[0:NPr, :],
                              mh[0:NPr, :], OP.mult)
            ml = scb.tile([128, 1024], F16, tag="scb")
            dve.tensor_scalar(ml[0:NPr, :], mb, LOWa, None, OP.is_ge)
            m1 = scb.tile([128, 1024], F16, tag="scb")
            dve.tensor_scalar(m1[0:NPr, :], mb, HIGHa, None, OP.is_le)
            mid = ded.tile([128, 1024], F16, tag="mid")
            gp.tensor_tensor(mid[0:NPr, :], ml[0:NPr, :], m1[0:NPr, :],
                             OP.mult)
            r3 = ded.tile([128, 1024], F16, tag="r3")
            dve.tensor_tensor(r3[0:NPr, :], hp[0:NPr, 0:1024],
                              hp[0:NPr, 2:1026], OP.add)
            dve.tensor_tensor(r3[0:NPr, :], r3[0:NPr, :],
                              hp[0:NPr, 1:1025], OP.add)
            # connect = [1,1,1]v @ r3 - hp  (f16 bands, integer-exact)
            c3p = psc.tile([128, 1024], F32, tag="psc")
            nc.tensor.matmul(c3p[0:NPr, 0:512], c3b[0:NPr, 0:NPr],
                             r3[0:NPr, 0:512], start=True, stop=True)
            nc.tensor.matmul(c3p[0:NPr, 512:1024], c3b[0:NPr, 0:NPr],
                             r3[0:NPr, 512:1024], start=True, stop=True)
            nc.tensor.matmul(c3p[0:NPr, 0:512], c3b[0:NPr, 128:128 + NPr],
                             hp[0:NPr, 1:513], start=False, stop=True)
            nc.tensor.matmul(c3p[0:NPr, 512:1024],
                             c3b[0:NPr, 128:128 + NPr],
                             hp[0:NPr, 513:1025], start=False, stop=True)
            # c3 is a nonneg integer; Sign(c3 - 0.5) = +1 iff c3 >= 1,
            # else -1 (negatives absorbed by the max with mh below)
            sgn = scb.tile([128, 1024], F16, tag="scb")
            act.activation(sgn[0:NPr, :], c3p[0:NPr, :], AF.Sign,
                           bias=auxt[0:NPr, 7:8])
            t_g = scb.tile([128, 1024], F16, tag="scb")
            dve.tensor_tensor(t_g[0:NPr, :], sgn[0:NPr, :], mid[0:NPr, :],
                              OP.mult)
            mx = scb.tile([128, 1024], F16, tag="scb")
            dve.tensor_tensor(mx[0:NPr, :], mh[0:NPr, :], t_g[0:NPr, :],
                              OP.max)
            # th = ((mx * im) * mT)
            th1 = scb.tile([128, 1024], F16, tag="scb")
            dve.tensor_tensor(th1[0:NPr, :], mx[0:NPr, :], im[0:NPr, :],
                              OP.mult)
            th = ded.tile([128, 1024], F16, tag="r3")
            dve.tensor_scalar(th[0:NPr, :], th1[0:NPr, :], mTa, None,
                              OP.mult)
            gp.memset(th[0:NPr, 0:1], 0.0)
            gp.memset(th[0:NPr, 1023:1024], 0.0)
            emit_store(th)

        # ================== window 0: 110 rows, per-image =================
        wst0, R0 = WINS[0]
        Rin0, R40 = R0 + 18, R0 + 4
        mM = auxt[0:R40, 4:5]
        bP = bPQ0[0:Rin0, 0:R40]
        bQ = bPQ0[0:Rin0, 114:114 + R40]

        m_t = [None] * B      # m maps [R40, 1026] f32 (col 0 / 1025 zero)
        mu_t = [None] * B
        md_t = [None] * B
        gxs_t = [None] * B
        gys_t = [None] * B

        for b in range(B):
            gxa = grp.tile([128, 1024], F32, tag=f"gxa{b}")
            gya = grp.tile([128, 1024], F32, tag=f"gya{b}")
            mt = mmp.tile([128, 1026], F32, tag=f"m{b}")
            if pad_once(f"m{b}", 1):
                gp.memset(mt[:, 0:1], 0.0)
                gp.memset(mt[:, 1025:1026], 0.0)
            sprev = None
            gxc = []
            gyc = []
            for c in range(C):
                hb = phase_a(b * C + c, wst0, Rin0)
                pq = psb.tile([128, 1024], F32, tag="pq")
                h1 = hb[0:Rin0, 1:513]
                h2_ = hb[0:Rin0, 513:1025]
                nc.tensor.matmul(pq[0:R40, 0:512], bP, h1,
                                 start=True, stop=True)
                nc.tensor.matmul(pq[0:R40, 512:1024], bP, h2_,
                                 start=True, stop=True)
                qq = psb.tile([128, 1024], F32, tag="pq")
                nc.tensor.matmul(qq[0:R40, 0:512], bQ, h1,
                                 start=True, stop=True)
                nc.tensor.matmul(qq[0:R40, 512:1024], bQ, h2_,
                                 start=True, stop=True)
                gxt, gyt, sprev = grad_mag(R40, pq, qq, mM, sprev, c,
                                           mt[0:R40, 1:1025])
                gxc.append(gxt)
                gyc.append(gyt)
            grad_sums(R40, gxa, gya, gxc, gyc)
            m_t[b] = mt
            gxs_t[b], gys_t[b] = gxa, gya

            # ---- mU / mD row shifts via SBUF->SBUF DMA ----
            # mu[r] = m[r+1] for r < R40-1; mu[R40-1] / md[0] are stale
            # garbage — they only feed NMS rows whose outputs are never
            # stored (th rows [2, 2+R0)), and is_gt of junk yields 0/1.
            mu = mmp.tile([128, 1026], F32, tag=f"mu{b}")
            nc.sync.dma_start(mu[0:R40 - 1, :], mt[1:R40, :])
            md = mmp.tile([128, 1026], F32, tag=f"md{b}")
            act.dma_start(md[1:R40, :], mt[0:R40 - 1, :])
            mu_t[b], md_t[b] = mu, md

        # -------------------- window 0 NMS (per image) -------------------
        for b in range(B):
            dy, dx = DIRS[b]

            def get_cmp(i, sg, dy=dy, dx=dx):
                src_ = {0: m_t, 1: mu_t, -1: md_t}[sg * dy][i]
                return (m_t[i][0:R40, 1:1025],
                        src_[0:R40, 1 + sg * dx:1 + sg * dx + 1024])

            def store(th, b=b):
                nc.sync.dma_start(out[b, wst0:wst0 + R0, 0:1024],
                                  th[2:2 + R0, 0:1024])

            nms(R40, gxs_t[b], gys_t[b], get_cmp, m_t[b][0:R40, 1:1025],
                auxt[0:R40, 0:1], auxt[0:R40, 1:2], auxt[0:R40, 2:3], bC3,
                store)

        # ========== window 1 convs: last 18 rows, 4 images packed ========
        # Emitted BEFORE the window-0 NMS so the in-order PE queue overlaps
        # this conv work with the DVE-heavy NMS phase.
        wst1, R1 = WINS[1]
        Rin, R4 = R1 + 18, R1 + 4
        NP = 96 + R4                      # 118 used partitions
        bP1 = bPQ1[0:Rin, 0:R4]
        bQ1 = bPQ1[0:Rin, 22:22 + R4]

        # m_p gap rows (32b+22..32b+31) come out exactly 0: the packed mM
        # scale is 0 there and the psum garbage under it is finite.
        m_p = mmp.tile([128, 1026], F32, tag="m0")
        sprev = None
        gxc = []
        gyc = []
        for c in range(C):
            pqP = psb.tile([128, 1024], F32, tag="pq")
            qqP = psb.tile([128, 1024], F32, tag="pq")
            s3 = None
            for b in range(B):
                hb = phase_a(b * C + c, wst1, Rin)
                h1 = hb[0:Rin, 1:513]
                h2_ = hb[0:Rin, 513:1025]
                if b < 3:
                    o = 32 * b
                    nc.tensor.matmul(pqP[o:o + R4, 0:512], bP1, h1,
                                     start=True, stop=True)
                    nc.tensor.matmul(pqP[o:o + R4, 512:1024], bP1, h2_,
                                     start=True, stop=True)
                    nc.tensor.matmul(qqP[o:o + R4, 0:512], bQ1, h1,
                                     start=True, stop=True)
                    nc.tensor.matmul(qqP[o:o + R4, 512:1024], bQ1, h2_,
                                     start=True, stop=True)
                else:
                    # PE out base must be 0/32/64: block 3 (partitions 96+)
                    # goes through its own psum tile + SBUF, then a DMA
                    # bounce into the packed p/q tiles.
                    p3 = psc.tile([128, 1024], F32, tag="psc")
                    nc.tensor.matmul(p3[0:R4, 0:512], bP1, h1,
                                     start=True, stop=True)
                    nc.tensor.matmul(p3[0:R4, 512:1024], bP1, h2_,
                                     start=True, stop=True)
                    nc.tensor.matmul(p3[32:32 + R4, 0:512], bQ1, h1,
                                     start=True, stop=True)
                    nc.tensor.matmul(p3[32:32 + R4, 512:1024], bQ1, h2_,
                                     start=True, stop=True)
                    s3 = sch.tile([128, 1024], F32, tag="scrf")
                    act.copy(s3[0:R4, :], p3[0:R4, :])
                    act.copy(s3[32:32 + R4, :], p3[32:32 + R4, :])

            def fixup(psb_, qsb, s3=s3):
                nc.sync.dma_start(psb_[96:96 + R4, 1:1025], s3[0:R4, :])
                nc.sync.dma_start(qsb[96:96 + R4, 1:1025],
                                  s3[32:32 + R4, :])

            gxt, gyt, sprev = grad_mag(NP, pqP, qqP, auxt[0:NP, 5:6],
                                       sprev, c, m_p[0:NP, 1:1025], fixup)
            gxc.append(gxt)
            gyc.append(gyt)
        gxap = grp.tile([128, 1024], F32, tag="gxa0")
        gyap = grp.tile([128, 1024], F32, tag="gya0")
        grad_sums(NP, gxap, gyap, gxc, gyc)

        # cross-image compare operands, built by partition/col-shifted DMAs:
        # rep[i] = m_i replicated into every 32-block; rhs[(i, s)] block b =
        # m_i shifted by s*d_b. Over/underflowing edge rows stay stale —
        # they only feed unstored output rows.
        rep = []
        for i in range(B):
            rp = mmp.tile([128, 1026], F32, tag=f"mu{i}")
            for b in range(B):
                qe = nc.sync if (i + b) % 2 == 0 else act
                qe.dma_start(rp[32 * b:32 * b + R4, 1:1025],
                             m_p[32 * i:32 * i + R4, 1:1025])
            rep.append(rp)
        rhs = {}
        rtags = {(0, 1): "md0", (1, 1): "md1", (2, 1): "md2",
                 (3, 1): "md3", (0, -1): "m1", (1, -1): "m2",
                 (2, -1): "m3"}
        for i in range(B):
            for s in (1, -1):
                if (i, s) in rtags:
                    rt = mmp.tile([128, 1026], F32, tag=rtags[(i, s)])
                else:
                    rt = pqp.tile([128, 1026], F32, tag="pqs")
                for b in range(B):
                    dyb, dxb = DIRS[b]
                    sp_ = 32 * i + s * dyb
                    lo = max(sp_, 0)
                    hi = min(sp_ + R4, 128)
                    qe = nc.sync if (i + b + (s > 0)) % 2 == 0 else act
                    qe.dma_start(
                        rt[32 * b + (lo - sp_):32 * b + (hi - sp_), 1:1025],
                        m_p[lo:hi, 1 + s * dxb:1025 + s * dxb])
                rhs[(i, s)] = rt

        def get_cmp_p(i, sg):
            return (rep[i][0:NP, 1:1025], rhs[(i, sg)][0:NP, 1:1025])

        def store_p(th):
            for b in range(B):
                nc.sync.dma_start(out[b, wst1:wst1 + R1, 0:1024],
                                  th[32 * b + 2:32 * b + 2 + R1, 0:1024])

        nms(NP, gxap, gyap, get_cmp_p, m_p[0:NP, 1:1025],
            auxt[0:NP, 0:1], auxt[0:NP, 1:2], auxt[0:NP, 3:4], bC3P,
            store_p)

    nc.compile()
    return nc


def _host_prep(img, gauss_h):
    """Build per-core inputs. Returns (in_maps, low, high)."""
    gh = np.asarray(gauss_h, np.float32).reshape(-1)

    flat = img.reshape(-1)
    r = (flat.size - 1) // 2
    v = np.partition(flat, r)[r]
    t1 = np.float32(max(np.float32(0.0),
                        np.float32(np.float32(0.7) * v)) * np.float32(6.0))
    t2 = np.float32(min(np.float32(1.0),
                        np.float32(np.float32(1.3) * v)) * np.float32(6.0))
    low = np.float32(min(t1, t2))
    high = np.float32(max(t1, t2))

    p = np.arange(128)[:, None]
    n = np.arange(128)[None, :]
    t = p - n
    bandHL = np.where((t >= 0) & (t <= 12), gh[np.clip(t, 0, 12)], 0.0
                      ).astype(np.float32)
    q12 = np.arange(12)[None, :]
    t12 = 12 + p - q12
    bandHR = np.where((t12 >= 0) & (t12 <= 12), gh[np.clip(t12, 0, 12)], 0.0
                      ).astype(np.float32)

    t5 = np.arange(128)[:, None] - n
    c111 = np.where(np.abs(t5) <= 1, 1.0, 0.0).astype(np.float32)
    negI = np.where(t5 == 0, -1.0, 0.0).astype(np.float32)
    bandC3 = np.concatenate([c111, negI], axis=1).astype(np.float16)
    # packed variant: [1,1,1] only within each 32-block's first 22 rows
    blk = np.arange(128) // 32
    rw = np.arange(128) % 32
    sameb = (blk[:, None] == blk[None, :])
    vq = (rw < 22)
    c111p = np.where(sameb & (np.abs(t5) <= 1)
                     & vq[:, None] & vq[None, :], 1.0, 0.0)
    bandC3P = np.concatenate([c111p, negI], axis=1).astype(np.float16)

    padded = np.zeros((B, C, H + 2 * HALO, W), np.float32)
    padded[:, :, HALO:HALO + H, :] = img

    w121 = np.array([1.0, 2.0, 1.0], np.float32)
    w101 = np.array([1.0, 0.0, -1.0], np.float32)

    in_maps = []
    for k in range(NCORES):
        slab = padded[:, :, BAND * k:BAND * k + SLABR, :]  # [B, C, SLABR, W]
        xT = np.zeros((B * C, WP, SLABR), np.float32)
        xT[:, 6:6 + W, :] = slab.reshape(B * C, SLABR, W).transpose(0, 2, 1)
        aux = np.zeros((128, 8), np.float32)
        aux[:, 0] = low
        aux[:, 1] = high
        aux[:, 7] = -0.5
        pq = []
        for wi, (wst, R) in enumerate(WINS):
            Rin, R4, R6 = R + 18, R + 4, R + 6
            g0 = BAND * k + wst
            maskBV = np.array([1.0 if 0 <= g0 - 3 + i < H else 0.0
                               for i in range(R6)], np.float32)
            for i in range(R4):
                mMv = 1.0 if 0 <= g0 - 2 + i < H else 0.0
                gr = g0 - 2 + i
                mTv = 0.0 if (gr == 0 or gr == H - 1) else 1.0
                if wi == 0:
                    aux[i, 4] = mMv
                    aux[i, 2] = mTv
                else:       # packed layout: same values in every 32-block
                    for b in range(B):
                        aux[32 * b + i, 5] = mMv
                        aux[32 * b + i, 3] = mTv
            # bandP[p, m] = sum_t w121[t] * maskBV[m+t] * gv[p-m-t]
            bP = np.zeros((Rin, R4), np.float32)
            bQ = np.zeros((Rin, R4), np.float32)
            pp_ = np.arange(Rin)[:, None]
            mm_ = np.arange(R4)[None, :]
            for ti in range(3):
                idx = pp_ - mm_ - ti
                gvv = np.where((idx >= 0) & (idx <= 12),
                               gh[np.clip(idx, 0, 12)], 0.0)
                bP += np.float32(w121[ti]) * maskBV[None, mm_[0] + ti] * gvv
                bQ += np.float32(w101[ti]) * maskBV[None, mm_[0] + ti] * gvv
            pq.append((bP.astype(np.float32), bQ.astype(np.float32)))
        b0 = np.zeros((128, 228), np.float32)
        b0[:, 0:114] = pq[0][0]
        b0[:, 114:228] = pq[0][1]
        b1 = np.zeros((36, 44), np.float32)
        b1[:, 0:22] = pq[1][0]
        b1[:, 22:44] = pq[1][1]
        in_maps.append({"xT": xT, "bandHL": bandHL, "bandHR": bandHR,
                        "bandPQ0": b0, "bandPQ1": b1,
                        "bandC3": bandC3, "bandC3P": bandC3P, "aux": aux})
    return in_maps, low, high


def kernel(img, gauss_h, gauss_v, sobel_h, sobel_v, dir_f, conn_f):
    from concourse import bass_utils

    img = np.ascontiguousarray(np.asarray(img, np.float32))
    in_maps, low, high = _host_prep(img, gauss_h)

    if "nc" not in _cache:
        _cache["nc"] = _build()
    nc = _cache["nc"]

    res = bass_utils.run_bass_kernel_spmd(
        nc, in_maps, core_ids=list(range(NCORES)))
    outs = [np.asarray(res.results[k]["out"], np.float32)
            for k in range(NCORES)]
    full = np.concatenate(outs, axis=1)          # [B, H, W]
    return full[:, None, :, :].astype(np.float32)
